# revision 85
# baseline (speedup 1.0000x reference)
import sys
sys.path.insert(0, "/opt/trn_rl_repo")
import numpy as np

N_ATOMS = 10000
N_SPECIES = 8
N_STRUCT = 8
C = 16
N_BASIS = 8
L_MAX = 3
CUTOFF = 5.0
NCORES = 8
NC_AT = N_ATOMS // NCORES
GRP = 5    # kp (chunk pairs) per PSUM batch
NBLK = 3   # ch pipeline blocks (geometry overlaps chunk math)

F16 = np.float16
_prog_cache = {}


def _pack(senders, receivers):
    send = np.asarray(senders).astype(np.int64)
    recv = np.asarray(receivers).astype(np.int64)
    order = np.argsort(recv, kind="stable")
    ss = send[order]
    deg = np.bincount(recv, minlength=N_ATOMS)
    CNT = None
    for c in (8, 7, 6, 5, 4, 3, 2):
        ok = True
        npd = ((NC_AT + c - 1) // c) * c
        for core in range(NCORES):
            d = np.zeros(npd, np.int64)
            d[:NC_AT] = deg[core * NC_AT:(core + 1) * NC_AT]
            if d.reshape(-1, c).sum(1).max() > 128:
                ok = False
                break
        if ok:
            CNT = c
            break
    assert CNT is not None
    NCH = (NC_AT + CNT - 1) // CNT
    quant = 2 * GRP * 3
    NCHE = ((NCH + quant - 1) // quant) * quant
    starts = np.zeros(N_ATOMS + 1, np.int64)
    starts[1:] = np.cumsum(deg)
    cores = []
    for core in range(NCORES):
        slot_send = np.zeros((128, NCHE), np.int64)
        mask = np.zeros((128, NCHE, CNT), np.float32)
        valid = np.zeros((128, NCHE), bool)
        for k in range(NCH):
            row = 0
            for j in range(CNT):
                r = core * NC_AT + k * CNT + j
                if r >= (core + 1) * NC_AT:
                    continue
                a, b = starts[r], starts[r + 1]
                n = b - a
                slot_send[row:row + n, k] = ss[a:b]
                mask[row:row + n, k, j] = 1.0
                valid[row:row + n, k] = True
                row += n
            assert row <= 128
        cores.append(dict(slot_send=slot_send, mask=mask, valid=valid))
    return CNT, NCH, NCHE, cores


def _build(CNT, NCHE, with_geom):
    """with_geom=True: launch A (geometry + layer1, writes PT + h1).
    False: launch B (loads PT, layer2, writes e row)."""
    import concourse.bass as bass
    import concourse.bacc as bacc
    import concourse.tile as tile
    from concourse import mybir

    f32 = mybir.dt.float32
    f16 = mybir.dt.float16
    ALU = mybir.AluOpType
    AF = mybir.ActivationFunctionType

    NPAD = CNT * NCHE
    NKP = NCHE // 2
    F = CNT * 16
    GCH2 = 2 * GRP
    PCH = GCH2 * 3 if NCHE % (GCH2 * 3) == 0 else GCH2
    NPP = NCHE // PCH        # PT dram pieces
    # uneven pipeline blocks: front-load geometry so the last block's
    # chunk tail is short
    pieces = NCHE // PCH
    b0 = max(1, pieces // 7)          # small first: chunks start early
    b2 = max(1, (2 * pieces) // 7)    # small-ish last: short tail
    b1 = pieces - b0 - b2
    BCHS = [b0 * PCH, b1 * PCH, b2 * PCH]
    BOFF = [0, BCHS[0], BCHS[0] + BCHS[1]]
    BMAX = max(BCHS)
    NG = NKP // GRP          # psum groups total
    GCH = 2 * GRP            # chunks per group

    # W-apply kp blocks aligned to pipeline block boundaries
    BLKS = [(BOFF[i] // 2, (BOFF[i] + BCHS[i]) // 2) for i in range(NBLK)]
    NGBs = [BCHS[i] // GCH for i in range(NBLK)]

    nc = bacc.Bacc("TRN2", target_bir_lowering=False, debug=False,
                   num_devices=NCORES)
    no_ = 16 if with_geom else 1
    OW = 64 + no_  # h0 rows at 0, h1 rows at partition 64 (legal base)
    SW_d = nc.dram_tensor("sw", [128, 16, OW], f16,
                          kind="ExternalInput").ap()
    if with_geom:
        GT_d = nc.dram_tensor("gt", [NG, 128, F, GCH], f16,
                              kind="ExternalInput").ap()
        PP_d = nc.dram_tensor("pp", [128, 6, NCHE], f32,
                              kind="ExternalInput").ap()
        WBX_d = nc.dram_tensor("wbx", [N_BASIS, 128, 16, BMAX], f16,
                               kind="ExternalInput").ap()
        CE_d = nc.dram_tensor("cemb", [16, NPAD], f32,
                              kind="ExternalInput").ap()
        PT_d = nc.dram_tensor("pt", [NPP, 128, 64, PCH], f16,
                              kind="ExternalOutput").ap()
        H1_d = nc.dram_tensor("h1", [16, NPAD], f32,
                              kind="ExternalOutput").ap()
    else:
        PT_d = nc.dram_tensor("pt", [NPP, 128, 64, PCH], f16,
                              kind="ExternalInput").ap()
        MST_d = nc.dram_tensor("mst", [128, CNT, NCHE], f16,
                               kind="ExternalInput").ap()
        HS_d = nc.dram_tensor("hst", [128, 16, NCHE], f16,
                              kind="ExternalInput").ap()
        CW_d = nc.dram_tensor("cw", [1, NPAD], f32,
                              kind="ExternalInput").ap()
        OE_d = nc.dram_tensor("oute", [1, NPAD], f32,
                              kind="ExternalOutput").ap()

    with tile.TileContext(nc) as tc:
        with tc.tile_pool(name="main", bufs=1) as pool, \
             tc.tile_pool(name="ppp", bufs=1) as ppool, \
             tc.tile_pool(name="wbp", bufs=2) as wbp, \
             tc.tile_pool(name="gp", bufs=4) as gpool, \
             tc.tile_pool(name="asp", bufs=4) as apool, \
             tc.tile_pool(name="pa", bufs=3, space="PSUM") as ppa, \
             tc.tile_pool(name="ph", bufs=2, space="PSUM") as pph:
            TT = nc.vector.tensor_tensor
            TS = nc.vector.tensor_scalar
            STT = nc.vector.scalar_tensor_tensor

            if not with_geom:
                MST = pool.tile([128, CNT, NCHE], f16, tag="mst")
                HST = pool.tile([128, 16, NCHE], f16, tag="hst")
                nc.sync.dma_start(MST[:], MST_d[:])
                nc.sync.dma_start(HST[:], HS_d[:])
            PPts = []
            if with_geom:
                for i in range(NBLK):
                    pp_t = ppool.tile([128, 6, BCHS[i]], f32, tag=f"pp{i}",
                                      name=f"pp{i}")
                    nc.sync.dma_start(
                        pp_t[:], PP_d[:, :, BOFF[i]:BOFF[i] + BCHS[i]])
                    PPts.append(pp_t)
            SW = pool.tile([128, 16, OW], f16, tag="sw")
            nc.sync.dma_start(SW[:], SW_d[:])
            PTTs = []
            for i in range(NPP):
                ptt_t = pool.tile([128, 64, PCH], f16, tag=f"ptt{i}",
                                  name=f"ptt{i}")
                PTTs.append(ptt_t)

            def pt_pieces(blk):
                off = BOFF[blk] // PCH
                return [(off + i, i * PCH) for i in range(BCHS[blk] // PCH)]

            if not with_geom:
                for i in range(NPP):
                    nc.sync.dma_start(PTTs[i][:], PT_d[i])

            if with_geom:
                CE = pool.tile([16, NPAD], f32, tag="ce")
                nc.sync.dma_start(CE[:], CE_d[:])
                # radial weights are constant along ch: 8 small persistent
                # tiles loaded once, reused by every block
                WBs = []
                for b in range(N_BASIS):
                    wb_t = pool.tile([128, 16, BMAX], f16, tag=f"wb{b}",
                                     name=f"wb{b}")
                    nc.sync.dma_start(wb_t[:], WBX_d[b])
                    WBs.append(wb_t)
                SC = pool.tile([128, 12, BMAX], f32, tag="sc")
                GSC = pool.tile([128, 8, BMAX], f32, tag="gsc")
                SHT = pool.tile([128, 16, BMAX], f16, tag="sht")
                T8 = pool.tile([128, 8, BMAX], f32, tag="t8")
                T8H = pool.tile([128, 8, BMAX], f16, tag="t8h")
                RRT = pool.tile([128, 16, BMAX], f16, tag="rrt")
                TMX = pool.tile([128, 16, BMAX], f16, tag="tmx")
                TMY = pool.tile([128, 16, BMAX], f16, tag="tmy")
                TMZ = pool.tile([128, 16, BMAX], f16, tag="tmz")
                RV = pool.tile([128, 3, BMAX], f32, tag="rv")
                U = pool.tile([128, 3, BMAX], f32, tag="u")
            else:
                CW = pool.tile([1, NPAD], f32, tag="cw")
                nc.sync.dma_start(CW[:], CW_d[:])

            if with_geom:
                GEOs = []
                for i in range(NBLK):
                    geo_t = pool.tile([128, 6, BCHS[i]], f32, tag=f"geo{i}",
                                      name=f"geo{i}")
                    GEOs.append(geo_t)
                PSC = pool.tile([128, 4, BMAX], f32, tag="psc")

            def geo_prefix(blk):
                """Act-engine-heavy part (sqrt/sin): runs early so these
                short ops are not queued behind the chunk loop's Act work.
                Output GEO rows: 0-2 = u, 3 = g, 4 = sin(th-pi),
                5 = -2*sin(th-pi/2) = 2cos(th)."""
                PP = PPts[blk]
                GEO = GEOs[blk]
                W = BCHS[blk]
                RVw = RV[:, :, 0:W]

                def ps(i):
                    return PSC[:, i, 0:W]

                TT(RVw, PP[:, 3:6, :], PP[:, 0:3, :], ALU.subtract)
                TT(GEO[:, 0:3, :], RVw, RVw, ALU.mult)
                TT(ps(0), GEO[:, 0, :], GEO[:, 1, :], ALU.add)
                TT(ps(0), ps(0), GEO[:, 2, :], ALU.add)          # r2
                nc.scalar.activation(ps(1), ps(0), AF.Sqrt)      # r
                nc.vector.tensor_scalar_max(ps(2), ps(1), 1e-6)  # rc
                nc.vector.reciprocal(ps(3), ps(2))               # 1/rc
                TT(GEO[:, 0:3, :], RVw,
                   PSC[:, 3:4, 0:W].to_broadcast([128, 3, W]), ALU.mult)
                # fc = -0.5*sin(pi*min(r,5)/5 - pi/2) + 0.5
                nc.vector.tensor_scalar_min(ps(0), ps(1), CUTOFF)
                TS(ps(0), ps(0), float(np.pi / CUTOFF), float(-np.pi / 2),
                   ALU.mult, ALU.add)
                nc.scalar.activation(ps(1), ps(0), AF.Sin)
                TS(ps(1), ps(1), -0.5, 0.5, ALU.mult, ALU.add)   # fc
                STT(GEO[:, 3, :], ps(1), float(np.sqrt(2.0 / CUTOFF)),
                    ps(3), ALU.mult, ALU.mult)                   # g
                TS(ps(0), ps(2), float(np.pi / CUTOFF), float(-np.pi),
                   ALU.mult, ALU.add)
                nc.scalar.activation(GEO[:, 4, :], ps(0), AF.Sin)
                TS(ps(0), ps(2), float(np.pi / CUTOFF), float(-np.pi / 2),
                   ALU.mult, ALU.add)
                nc.scalar.activation(ps(1), ps(0), AF.Sin)
                nc.vector.tensor_scalar_mul(GEO[:, 5, :], ps(1), -2.0)

            def geom_block(blk):
                GEO = GEOs[blk]
                W = BCHS[blk]

                def sc(i):
                    return SC[:, i, 0:W]

                x, y, z = GEO[:, 0, :], GEO[:, 1, :], GEO[:, 2, :]
                GP = nc.gpsimd

                def tm(dst, b, eng=None):
                    t8b = T8H[:, b:b + 1, 0:W].to_broadcast([128, 16, W])
                    (eng or nc.vector).tensor_tensor(
                        dst[:, :, 0:W], WBs[b][:, :, 0:W], t8b, ALU.mult)

                # b=0,1 basis values exist immediately; gp computes their TM
                # products (consumed by the LAST adds) before its SH work
                c2 = GEO[:, 5, :]
                TT(T8[:, 0, 0:W], GEO[:, 3, :], GEO[:, 4, :], ALU.mult)
                TT(T8[:, 1, 0:W], c2, T8[:, 0, 0:W], ALU.mult)
                GP.tensor_scalar_mul(T8H[:, 0:2, 0:W], T8[:, 0:2, 0:W], 1.0)
                tm(TMX, 0, GP)
                tm(TMZ, 1, GP)

                def gs(i):
                    return GSC[:, i, 0:W]

                x2, y2, z2, xy, yz, xz = (gs(i) for i in range(6))
                t_, d_ = gs(6), gs(7)
                GP.tensor_tensor(x2, x, x, ALU.mult)
                GP.tensor_tensor(y2, y, y, ALU.mult)
                GP.tensor_tensor(z2, z, z, ALU.mult)
                GP.tensor_tensor(xy, x, y, ALU.mult)
                GP.tensor_tensor(yz, y, z, ALU.mult)
                GP.tensor_tensor(xz, x, z, ALU.mult)

                def sh(m):
                    return SHT[:, m, 0:W]

                GP.memset(sh(0), 0.28209479)
                GP.tensor_scalar_mul(sh(1), y, 0.48860251)
                GP.tensor_scalar_mul(sh(2), z, 0.48860251)
                GP.tensor_scalar_mul(sh(3), x, 0.48860251)
                GP.tensor_scalar_mul(sh(4), xy, 1.09254843)
                GP.tensor_scalar_mul(sh(5), yz, 1.09254843)
                TS(sh(6), z2, 3.0 * 0.31539157, -0.31539157,
                   ALU.mult, ALU.add)
                GP.tensor_scalar_mul(sh(7), xz, 1.09254843)
                GP.tensor_tensor(d_, x2, y2, ALU.subtract)
                GP.tensor_scalar_mul(sh(8), d_, 0.54627422)
                STT(t_, x2, 3.0, y2, ALU.mult, ALU.subtract)
                GP.tensor_tensor(t_, t_, y, ALU.mult)
                GP.tensor_scalar_mul(sh(9), t_, 0.59004359)
                GP.tensor_tensor(t_, xy, z, ALU.mult)
                GP.tensor_scalar_mul(sh(10), t_, 2.89061144)
                TS(t_, z2, 5.0 * 0.45704579, -0.45704579, ALU.mult,
                   ALU.add)
                GP.tensor_tensor(sh(11), t_, y, ALU.mult)
                GP.tensor_tensor(sh(13), t_, x, ALU.mult)
                TS(t_, z2, 5.0 * 0.37317633, -3.0 * 0.37317633,
                   ALU.mult, ALU.add)
                GP.tensor_tensor(sh(12), t_, z, ALU.mult)
                GP.tensor_tensor(t_, d_, z, ALU.mult)
                GP.tensor_scalar_mul(sh(14), t_, 1.44530572)
                STT(t_, y2, -3.0, x2, ALU.mult, ALU.add)
                GP.tensor_tensor(t_, t_, x, ALU.mult)
                GP.tensor_scalar_mul(sh(15), t_, 0.59004359)

                # T_b = -g*sin(b*theta): sign cancels in A^2
                for b in range(2, N_BASIS):
                    TT(sc(9) if b % 2 else sc(1), c2, T8[:, b - 1, 0:W],
                       ALU.mult)
                    TT(T8[:, b, 0:W], sc(9) if b % 2 else sc(1),
                       T8[:, b - 2, 0:W], ALU.subtract)
                nc.vector.tensor_scalar_mul(T8H[:, 2:8, 0:W],
                                            T8[:, 2:8, 0:W], 1.0)

                # RR[slot, ln, ch] = sum_b WRB[b,ln] * T8[slot,b,ch]
                tm(TMY, 2)
                tm(RRT, 3)
                TT(TMY[:, :, 0:W], TMY[:, :, 0:W], RRT[:, :, 0:W], ALU.add)
                tm(RRT, 4)
                TT(TMY[:, :, 0:W], TMY[:, :, 0:W], RRT[:, :, 0:W], ALU.add)
                tm(RRT, 5)
                TT(TMY[:, :, 0:W], TMY[:, :, 0:W], RRT[:, :, 0:W], ALU.add)
                tm(RRT, 6)
                TT(TMY[:, :, 0:W], TMY[:, :, 0:W], RRT[:, :, 0:W], ALU.add)
                tm(RRT, 7)
                TT(TMY[:, :, 0:W], TMY[:, :, 0:W], RRT[:, :, 0:W], ALU.add)
                TT(TMX[:, :, 0:W], TMX[:, :, 0:W], TMZ[:, :, 0:W], ALU.add)
                TT(RRT[:, :, 0:W], TMX[:, :, 0:W], TMY[:, :, 0:W], ALU.add)

                # PT[slot, (m,n), ch] = SH[slot,m,ch] * RR[slot,(l,n),ch]
                for (pi_, loc) in pt_pieces(blk):
                    for l in range(L_MAX + 1):
                        nm = 2 * l + 1
                        shv = SHT[:, l * l:l * l + nm, loc:loc + PCH] \
                            .unsqueeze(2).to_broadcast([128, nm, 4, PCH])
                        rrv = RRT[:, 4 * l:4 * l + 4, loc:loc + PCH] \
                            .unsqueeze(1).to_broadcast([128, nm, 4, PCH])
                        ptv = PTTs[pi_][:, 4 * l * l:4 * (l * l + nm), :] \
                            .rearrange("p (m n) c -> p m n c", n=4)
                        TT(ptv, shv, rrv, ALU.mult)

            # fused invariant projection: for each group and channel c,
            # accumulate SW[h,c].T @ As[:, :, :, c] straight into the
            # per-(block,h) psum output region (exact f32, no INV staging)
            if with_geom:
                OUTH = pool.tile([16, NPAD], f32, tag="outh")
            else:
                OUTE = pool.tile([1, NPAD], f32, tag="oute1")
            _phs = {}

            def blk_of(q):
                b = 0
                while q * GCH >= BOFF[b] + BCHS[b]:
                    b += 1
                return b

            def chunk_group(q):
                blk = blk_of(q)
                qloc = (q * GCH - BOFF[blk]) // GCH
                if qloc == 0:
                    _phs[blk] = pph.tile([OW, 512], f32, tag="ph",
                                         name=f"ph_{blk}")
                GTg = gpool.tile([128, F, GCH], f16, tag="gtg")
                if with_geom:
                    nc.sync.dma_start(GTg[:], GT_d[q])
                else:
                    c0 = q * GCH
                    hsv = HST[:, :, c0:c0 + GCH].unsqueeze(1) \
                        .to_broadcast([128, CNT, 16, GCH])
                    msv = MST[:, :, c0:c0 + GCH].unsqueeze(2) \
                        .to_broadcast([128, CNT, 16, GCH])
                    TT(GTg[:].rearrange("p (j c) k -> p j c k", c=16),
                       hsv, msv, ALU.mult)
                pa = ppa.tile([128, GRP * F], f32, tag="pa")
                for kpl in range(GRP):
                    kp = q * GRP + kpl
                    for h in (0, 1):
                        k = 2 * kp + h
                        nc.tensor.matmul(
                            pa[64 * h:64 * (h + 1), kpl * F:(kpl + 1) * F],
                            PTTs[k // PCH][:, :, k % PCH],
                            GTg[:, :, 2 * kpl + h],
                            start=True, stop=True)
                As = apool.tile([128, GRP * F], f16, tag="as")
                nc.scalar.activation(As[:], pa[:], AF.Square)
                Asv = As[:].rearrange("p (k j c) -> p k j c", c=16, j=CNT)
                ncg = GRP * CNT
                ph = _phs[blk]
                for c in range(16):
                    nc.tensor.matmul(
                        ph[:, qloc * ncg:(qloc + 1) * ncg],
                        SW[:, c, :], Asv[:, :, :, c],
                        start=(c == 0), stop=(c == 15))

            def evac_block(blk):
                k0, k1 = BLKS[blk]
                ncol = (k1 - k0) * CNT
                for h in (0, 1):
                    ph = _phs[blk][64 * h:64 * h + no_, :]
                    # atom = 2*CNT*kp + CNT*h + j
                    if with_geom:
                        dst = OUTH[:].rearrange(
                            "p (k t j) -> p k t j", t=2, j=CNT)[:, k0:k1, h, :]
                        cev = CE[:].rearrange(
                            "p (k t j) -> p k t j", t=2, j=CNT)[:, k0:k1, h, :]
                        TT(dst, ph[:, 0:ncol].rearrange(
                            "p (k j) -> p k j", j=CNT), cev, ALU.mult)  # h-sliced
                    else:
                        dst = OUTE[:].rearrange(
                            "p (k t j) -> p k t j", t=2, j=CNT)[:, k0:k1, h, :]
                        cwv = CW[:].rearrange(
                            "p (k t j) -> p k t j", t=2, j=CNT)[:, k0:k1, h, :]
                        TT(dst, ph[0:1, 0:ncol].rearrange(
                            "p (k j) -> p k j", j=CNT), cwv, ALU.add)  # h-sliced

            if with_geom:
                for blk in range(NBLK):
                    geo_prefix(blk)
            q0 = 0
            for blk in range(NBLK):
                if with_geom:
                    geom_block(blk)
                for ql in range(NGBs[blk]):
                    chunk_group(q0 + ql)
                q0 += NGBs[blk]
                evac_block(blk)
                if with_geom:
                    for (pi_, loc) in pt_pieces(blk):
                        nc.sync.dma_start(PT_d[pi_], PTTs[pi_][:])

            if with_geom:
                nc.sync.dma_start(H1_d[:], OUTH[:])
            else:
                nc.sync.dma_start(OE_d[:], OUTE[:])
    nc.compile()
    return nc


def _sw_pack(W, wo):
    """Fused selection+projection weights: SW[p=(h*64+mi*4+n), h, c, o] =
    W[(l*4+n)*16+c, o] / sqrt(2l+1), nonzero only for the matching h."""
    no = 16 if wo is None else 1
    Waug = W if wo is None else wo[:, None]
    # out cols: h0 rows at 0..no, h1 rows at 64..64+no (legal psum base)
    SW = np.zeros((128, 16, 64 + no), np.float32)
    mi = 0
    for l in range(L_MAX + 1):
        s = 1.0 / np.sqrt(2.0 * l + 1.0)
        for m in range(2 * l + 1):
            for n in range(4):
                for h in (0, 1):
                    row = h * 64 + mi * 4 + n
                    for c in range(16):
                        SW[row, c, 64 * h:64 * h + no] = \
                            Waug[(l * 4 + n) * 16 + c] * s
            mi += 1
    return SW.astype(F16)


def kernel(positions, embed, W_rad, W_inv1, W_inv2, w_out, comp_weights,
           senders, receivers, species, structure_ids):
    from concourse import bass_utils

    positions = np.asarray(positions, np.float32)
    embed = np.asarray(embed, np.float32)
    W_rad = np.asarray(W_rad, np.float32)
    W_inv1 = np.asarray(W_inv1, np.float32)
    W_inv2 = np.asarray(W_inv2, np.float32)
    w_out = np.asarray(w_out, np.float32)
    comp_weights = np.asarray(comp_weights, np.float32)
    senders = np.asarray(senders).astype(np.int64)
    receivers = np.asarray(receivers).astype(np.int64)
    species = np.asarray(species).astype(np.int64)
    structure_ids_np = np.asarray(structure_ids).astype(np.int64)

    CNT, NCH, NCHE, cores = _pack(senders, receivers)
    NPAD = CNT * NCHE
    GCH = 2 * GRP
    PCH = GCH * 3 if (NCHE // NBLK) % (GCH * 3) == 0 else GCH
    pieces = NCHE // PCH
    b2 = max(1, pieces // 7)
    b0 = (pieces - b2 + 1) // 2
    BMAX = max(b0, pieces - b0 - b2) * PCH
    key = (CNT, NCHE)
    if key not in _prog_cache:
        _prog_cache[key] = (_build(CNT, NCHE, True), _build(CNT, NCHE, False))
    ncA, ncB = _prog_cache[key]

    cemb = embed[species]
    sw_1 = _sw_pack(W_inv1, None)
    sw_2 = _sw_pack(None, w_out)
    WRB = np.zeros((N_BASIS, 16), np.float32)
    for l in range(L_MAX + 1):
        WRB[:, l * 4:(l + 1) * 4] = W_rad[l]
    WBX = np.ascontiguousarray(np.broadcast_to(
        WRB.astype(F16)[:, None, :, None], (N_BASIS, 128, 16, BMAX)))

    mapsA, mapsB = [], []
    for core in range(NCORES):
        cd = cores[core]
        ss, msk, val = cd["slot_send"], cd["mask"], cd["valid"]
        rloc = msk.argmax(2)
        rglob = core * NC_AT + (np.arange(NCHE)[None, :] * CNT + rloc)
        rglob = np.clip(rglob, 0, N_ATOMS - 1)
        pp = np.zeros((128, NCHE, 6), np.float32)
        pp[:, :, 0:3] = np.where(val[:, :, None], positions[ss], 0.0)
        pp[:, :, 3:6] = np.where(val[:, :, None], positions[rglob], 0.0)
        ppT = np.ascontiguousarray(pp.transpose(0, 2, 1))
        at = np.arange(core * NC_AT, core * NC_AT + NPAD)
        atc = np.clip(at, 0, N_ATOMS - 1)
        apad = (at < (core + 1) * NC_AT)
        cemb_t = np.where(apad[None, :], cemb[atc].T, 0.0).astype(np.float32)
        cw_t = np.where(apad, comp_weights[species[atc]], 0.0
                        ).astype(np.float32)[None, :]
        # layer-1 G: mask (x) cemb[send], group-major with ch innermost
        hs0 = np.where(val[:, :, None], cemb[ss], 0.0)
        g1 = (msk[:, :, :, None] * hs0[:, :, None, :])  # [128,NCHE,CNT,16]
        NG = NCHE // (2 * GRP)
        g1T = g1.transpose(0, 2, 3, 1).reshape(128, CNT * 16, NG, 2 * GRP)
        g1T = np.ascontiguousarray(g1T.transpose(2, 0, 1, 3)).astype(F16)
        mapsA.append(dict(pp=ppT, gt=g1T, wbx=WBX, sw=sw_1,
                          cemb=np.ascontiguousarray(cemb_t)))
        mskT = np.ascontiguousarray(msk.transpose(0, 2, 1)).astype(F16)
        mapsB.append(dict(mst=mskT, sw=sw_2,
                          cw=np.ascontiguousarray(cw_t)))

    resA = bass_utils.run_bass_kernel_spmd(ncA, mapsA,
                                           core_ids=list(range(NCORES)))
    h_full = np.concatenate(
        [resA.results[c]["h1"][:, 0:NC_AT].T for c in range(NCORES)], 0)
    for core in range(NCORES):
        cd = cores[core]
        hsg = np.where(cd["valid"][:, :, None],
                       h_full[cd["slot_send"]], 0.0)
        mapsB[core]["hst"] = np.ascontiguousarray(
            hsg.transpose(0, 2, 1)).astype(F16)
        mapsB[core]["pt"] = resA.results[core]["pt"]

    resB = bass_utils.run_bass_kernel_spmd(ncB, mapsB,
                                           core_ids=list(range(NCORES)))
    e_atom = np.concatenate(
        [resB.results[c]["oute"][0, 0:NC_AT] for c in range(NCORES)], 0)
    out = np.zeros(N_STRUCT, np.float32)
    np.add.at(out, structure_ids_np, e_atom)
    return out
